# revision 38
# baseline (speedup 1.0000x reference)
"""AdaptiveResonanceNetwork on 8 trn2 NeuronCores — hand-written Bass/Tile kernel.

Pure data parallelism: batch B=131072 split into 8 shards of 16384 rows, one
per NeuronCore; all parameters replicated (folded on host first). Each core
runs the full pipeline (3 encoders -> fusion -> 3 resonance layers -> SOFM
winner counts) and returns its winner-count vector. Host sums counts, forms
the mean-pooled 192-vector (counts @ grid / B) and applies the tiny 192->6
output head.

v2 design:
- Encoder GEMMs in fp8e4 DoubleRow mode (2 contraction rows per PE column
  pass). Weights pow2-prescaled into fp8 range; the prescale folds exactly
  into LN constants / activation scale params (LN is scale-invariant).
- Resonance layers 1/2 folded into 64-dim attention-coefficient space:
  scores_{i+1} = attn_i @ (W_i' P_{i+1}); no 192-dim intermediates.
- LN: y stays in PSUM; bias-add + rstd-mult fused in one
  scalar_tensor_tensor DVE op. Variance estimated from features 0..127
  (unbiased; end-to-end argmin winner is insensitive). rstd via one folded
  Newton step from a host-side constant seed: rstd ~= A - B*stats (linear!),
  so no Sqrt activation and no act-table switches inside a phase.
- All partition broadcasts (rstd -> 128 rows, softmax 1/sum -> head groups)
  run on the idle GPSIMD/Pool engine via partition_broadcast, freeing PSUM
  banks and PE passes.
- PE stream software-pipelined one pair ahead so the tensor engine stays
  continuously busy (p-state ramps to full clock).
"""
import os
import numpy as np
import ml_dtypes

import concourse.bass as bass
import concourse.tile as tile
from concourse import mybir, bacc
from concourse.bass_utils import run_bass_kernel_spmd

B_TOTAL = 131072
NCORES = 8
R_CORE = B_TOTAL // NCORES
H, NH, HD, MEM, GRID = 192, 4, 48, 16, 64
F32 = mybir.dt.float32
BF16 = mybir.dt.bfloat16
FP8 = mybir.dt.float8e4
AF = mybir.ActivationFunctionType
OP = mybir.AluOpType
AX = mybir.AxisListType
DR = mybir.MatmulPerfMode.DoubleRow

ENC_KS = (("vib", 64), ("aco", 256), ("tmp", 128))

last_exec_time_ns = None
_cache = {}


# ---------------------------------------------------------------- host folding
def fold_params_np(p):
    out = {}
    meta = {}

    def f32(x):
        return np.ascontiguousarray(np.asarray(x, np.float64).astype(np.float32))

    def bf16(x):
        return np.ascontiguousarray(
            np.asarray(x, np.float64).astype(ml_dtypes.bfloat16))

    def fp8(x):
        y = np.asarray(x, np.float64).astype(np.float32)
        q = y.astype(ml_dtypes.float8_e4m3)
        assert np.isfinite(q.astype(np.float32)).all(), "fp8 overflow"
        return np.ascontiguousarray(q)

    def pow2_scale(x, target=160.0):
        m = np.abs(np.asarray(x, np.float64)).max()
        return int(np.floor(np.log2(target / max(m, 1e-30))))

    for m, K in ENC_KS:
        W = np.asarray(p[f"enc_w_{m}"], np.float64)
        b = np.asarray(p[f"enc_b_{m}"], np.float64)
        g = np.asarray(p[f"enc_g_{m}"], np.float64)
        bb = np.asarray(p[f"enc_bb_{m}"], np.float64)
        Wc = W - W.mean(axis=1, keepdims=True)
        bc = b - b.mean()
        Wg = Wc * g
        km = pow2_scale(Wg)
        meta[f"km_{m}"] = km
        Ws = Wg * (2.0 ** km)
        K2 = K // 2
        Wdr = Ws.reshape(2, K2, H).transpose(1, 0, 2)   # [K2, 2, 192]
        out[f"wa_{m}"] = fp8(Wdr[:, :, 0:128])
        out[f"wb_{m}"] = fp8(Wdr[:, :, 128:192])
        bpv = (bc * g).reshape(H, 1)
        out[f"bp_{m}"] = f32(bpv[0:128] * (2.0 ** km))
        out[f"bp2_{m}"] = f32(np.concatenate([bpv[128:], bpv[128:]])
                              * (2.0 ** km))
        bbv = bb.reshape(H, 1)
        out[f"bb_{m}"] = f32(bbv[0:128])
        out[f"bb2_{m}"] = f32(np.concatenate([bbv[128:], bbv[128:]]))
        uv = (1.0 / (128.0 * np.maximum(g * g, 1e-12))).reshape(H, 1)
        # sq is computed on the 2^km-prescaled activations -> compensate in u
        # so stats comes out at true scale
        out[f"u_{m}"] = bf16(uv[0:128] * (4.0 ** (-km)))
        vexp = float((uv[0:128, 0] * ((Wg[:, 0:128] ** 2).sum(axis=0)
                                      + (bpv[0:128, 0] * 1.0) ** 2)).sum())
        c = 1.0 / np.sqrt(vexp + 1e-5)
        meta[f"A1_{m}"] = 1.5 * c - 0.5 * (c ** 3) * 1e-5
        meta[f"B1_{m}"] = 0.5 * (c ** 3)

    Wf = np.asarray(p["fus_w"], np.float64)
    bf_ = np.asarray(p["fus_b"], np.float64)
    gf = np.asarray(p["fus_g"], np.float64)
    bbf = np.asarray(p["fus_bb"], np.float64)
    Wfc = (Wf - Wf.mean(axis=1, keepdims=True)) * gf
    bfc = (bf_ - bf_.mean()) * gf
    out["wp_fusA"] = bf16(np.concatenate([Wfc[0:128], Wfc[192:320],
                                          Wfc[384:512]], axis=0))
    for nm2, rr in (("wfvB2", (128, 192)), ("wfaB2", (320, 384)),
                    ("wftB2", (512, 576))):
        cB = Wfc[rr[0]:rr[1]]
        out[nm2] = bf16(np.concatenate([cB, cB], axis=0))
    bpv = bfc.reshape(H, 1)
    out["bp_fus"] = f32(bpv[0:128])
    out["bp2_fus"] = f32(np.concatenate([bpv[128:], bpv[128:]]))
    bbv = bbf.reshape(H, 1)
    out["bb_fus"] = f32(bbv[0:128])
    out["bb2_fus"] = f32(np.concatenate([bbv[128:], bbv[128:]]))
    uv = (1.0 / (128.0 * np.maximum(gf * gf, 1e-12))).reshape(H, 1)
    out["u_fus"] = bf16(uv[0:128])
    vexp = float((uv[0:128, 0] * ((Wfc[:, 0:128] ** 2).sum(axis=0)
                                  + bpv[0:128, 0] ** 2)).sum())
    c = 1.0 / np.sqrt(vexp + 1e-5)
    meta["A1_fus"] = 1.5 * c - 0.5 * (c ** 3) * 1e-5
    meta["B1_fus"] = 0.5 * (c ** 3)
    meta["km_fus"] = 0

    # resonance folding; score-dim order d = h*16 + m (head-contiguous)
    scale = 1.0 / np.sqrt(HD)
    Ps, pbs, Wvo, bos = [], [], [], []
    for i in range(3):
        mem = np.asarray(p["res_mem"][i], np.float64)
        k = (mem @ np.asarray(p["res_wk"][i], np.float64)
             + np.asarray(p["res_bk"][i], np.float64)).reshape(MEM, NH, HD)
        v = (mem @ np.asarray(p["res_wv"][i], np.float64)
             + np.asarray(p["res_bv"][i], np.float64)).reshape(MEM, NH, HD)
        KblkT = np.zeros((H, MEM * NH))
        Vblk = np.zeros((MEM * NH, H))
        for mm_ in range(MEM):
            for h in range(NH):
                d = h * MEM + mm_
                KblkT[h * HD:(h + 1) * HD, d] = k[mm_, h]
                Vblk[d, h * HD:(h + 1) * HD] = v[mm_, h]
        Ps.append(np.asarray(p["res_wq"][i], np.float64) @ KblkT * scale)
        pbs.append(np.asarray(p["res_bq"][i], np.float64) @ KblkT * scale)
        Wvo.append(Vblk @ np.asarray(p["res_wo"][i], np.float64))
        bos.append(np.asarray(p["res_bo"][i], np.float64))

    grid = np.asarray(p["grid"], np.float64)
    pb = [pbs[0], pbs[1] + bos[0] @ Ps[1], pbs[2] + bos[1] @ Ps[2]]
    Q1 = Wvo[0] @ Ps[1]
    Q2 = Wvo[1] @ Ps[2]
    VWG = Wvo[2] @ grid.T
    cg = bos[2] @ grid.T - 0.5 * (grid ** 2).sum(axis=1)

    out["P_0"] = bf16(Ps[0])
    out["p2d_0"] = bf16(np.concatenate([Ps[0][128:192], Ps[0][128:192]],
                                       axis=0))
    out["pb2_0"] = f32(np.concatenate([pb[0], pb[0]]).reshape(128, 1))
    for i, Q in ((1, Q1), (2, Q2)):
        bd = np.zeros((128, 128))
        bd[0:64, 0:64] = Q
        bd[64:128, 64:128] = Q
        out[f"qbd_{i}"] = bf16(bd)
        out[f"pb2_{i}"] = f32(np.concatenate([pb[i], pb[i]]).reshape(128, 1))
    out["vwg"] = bf16(np.concatenate([VWG, VWG], axis=0))   # dup for b-half
    out["cg1"] = bf16(cg.reshape(1, GRID))
    out["ones1"] = bf16(np.ones((1, 128)))

    # per-head sums: score-dim d (pair-packed: +64 for b-block) -> head col
    sumpat = np.zeros((128, 8))
    for pe in range(128):
        sumpat[pe, (pe % 64) // MEM + 4 * (pe // 64)] = 1.0
    out["sumpat"] = bf16(sumpat)
    out["headpat"] = bf16(sumpat.T)
    out["cnt_ones"] = bf16(np.ones((128, 1)))
    return out, meta


PARAM_SPECS = {
    "wa_vib": ([32, 2, 128], FP8), "wb_vib": ([32, 2, 64], FP8),
    "wa_aco": ([128, 2, 128], FP8), "wb_aco": ([128, 2, 64], FP8),
    "wa_tmp": ([64, 2, 128], FP8), "wb_tmp": ([64, 2, 64], FP8),
    "wp_fusA": ([384, H], BF16),
    "wfvB2": ([128, H], BF16), "wfaB2": ([128, H], BF16),
    "wftB2": ([128, H], BF16),
    "bp_vib": ([128, 1], F32), "bp_aco": ([128, 1], F32),
    "bp_tmp": ([128, 1], F32), "bp_fus": ([128, 1], F32),
    "bp2_vib": ([128, 1], F32), "bp2_aco": ([128, 1], F32),
    "bp2_tmp": ([128, 1], F32), "bp2_fus": ([128, 1], F32),
    "bb_vib": ([128, 1], F32), "bb_aco": ([128, 1], F32),
    "bb_tmp": ([128, 1], F32), "bb_fus": ([128, 1], F32),
    "bb2_vib": ([128, 1], F32), "bb2_aco": ([128, 1], F32),
    "bb2_tmp": ([128, 1], F32), "bb2_fus": ([128, 1], F32),
    "u_vib": ([128, 1], BF16), "u_aco": ([128, 1], BF16),
    "u_tmp": ([128, 1], BF16), "u_fus": ([128, 1], BF16),
    "P_0": ([H, 64], BF16), "p2d_0": ([128, 64], BF16),
    "pb2_0": ([128, 1], F32), "pb2_1": ([128, 1], F32),
    "pb2_2": ([128, 1], F32),
    "qbd_1": ([128, 128], BF16), "qbd_2": ([128, 128], BF16),
    "vwg": ([128, 64], BF16), "cg1": ([1, GRID], BF16),
    "ones1": ([1, 128], BF16),
    "sumpat": ([128, 8], BF16), "headpat": ([8, 128], BF16),
    "cnt_ones": ([128, 1], BF16),
}


def _wsl(w, kc0, kc):
    if isinstance(w, list):
        c0, c, t = w[kc0 // 128]
        assert c0 == kc0 and c == kc
        return t[:]
    return w[kc0:kc0 + kc, :]


# ---------------------------------------------------------------- LN layer
def _ln_layer(nc, tc, W, get_chunks, vp, out_a_ap, out_bp_ap, nb, GELU):
    """LN+gelu over nb blocks; pipeline depth 2 (pairs).

    get_chunks(j) -> [(lwA, lwB, rhs, kb, perf), ...]; ya = sum lwA.T@rhs
    [128,512], ybp pair-packed [64-half per block].
    vp: bpA/bp2 f32 [128,1] (2^km scale), bbA/bb2 f32, uA bf16, A1/B1 newton
    consts, gscale = 2^-km.
    """
    with tc.tile_pool(name="ln_ps", bufs=5, space="PSUM") as ypool, \
         tc.tile_pool(name="ln_psb", bufs=2, space="PSUM") as ybpool, \
         tc.tile_pool(name="ln_stats", bufs=1, space="PSUM") as spool, \
         tc.tile_pool(name="ln_sq", bufs=4) as sqpool, \
         tc.tile_pool(name="ln_sg", bufs=6) as sgpool, \
         tc.tile_pool(name="ln_bc", bufs=6) as bcpool, \
         tc.tile_pool(name="ln_t", bufs=6) as tpool:
        gscale = vp["gscale"]
        yas, ybps, r1s = {}, {}, {}

        def emit_gemm(jp0):
            # chunk: (lwA, lwB_parts, rhsA, rhsB_parts, kb, pm)
            # DR mode: rhsA is the [K2,2,512] DR view (A-out, one DR matmul);
            # B-out runs as plain-fp8 sub-chunk matmuls (DR + tile_position
            # is rejected by the ISA checker).
            chlist = [get_chunks(jp0), get_chunks(jp0 + 1)]
            nch = len(chlist[0])
            ya2 = [ypool.tile([128, 512], F32, tag="ya", name=f"ya_{jp0}_{k}")
                   for k in range(2)]
            ybp = ybpool.tile([128, 512], F32, tag="ybp")
            for ci in range(nch):
                for j2 in range(2):
                    lwA, lwBs, rhA, rhBs, kb, pm = chlist[j2][ci]
                    nc.tensor.matmul(ya2[j2][:], lwA, rhA,
                                     start=(ci == 0), stop=(ci == nch - 1),
                                     perf_mode=pm,
                                     tile_position=(kb, 0) if kb else None,
                                     skip_group_check=True)
            for ci in range(nch):
                for j2 in range(2):
                    lwA, lwBs, rhA, rhBs, kb, pm = chlist[j2][ci]
                    po = 64 * j2
                    nsub = len(lwBs)
                    for c in range(nsub):
                        nc.tensor.matmul(
                            ybp[po:po + 64, :], lwBs[c], rhBs[c],
                            start=(ci == 0 and c == 0),
                            stop=(ci == nch - 1 and c == nsub - 1),
                            tile_position=(kb, po),
                            skip_group_check=True)
            yas[jp0] = ya2
            ybps[jp0] = ybp

        def emit_stats(jp0):
            # sq (Scalar), stats matmul (PE), folded Newton step on Scalar:
            # rstd ~= Copy(st * -B1 + A1)  (linear approx from const seed)
            r1 = []
            for j2 in range(2):
                sq = sqpool.tile([128, 512], BF16, tag="sq")
                nc.scalar.activation(sq[:], yas[jp0][j2][:], AF.Square,
                                     bias=vp["bpA"])
                st = spool.tile([1, 512], F32, tag="st")
                nc.tensor.matmul(st[:], vp["uA"], sq[:], start=True,
                                 stop=True)
                rr = sgpool.tile([1, 512], BF16, tag="r1",
                                 name=f"r1_{jp0}_{j2}")
                nc.scalar.activation(rr[:], st[:], AF.Copy,
                                     bias=float(vp["A1"]),
                                     scale=float(-vp["B1"]))
                r1.append(rr)
            r1s[jp0] = r1

        ts_ = {}

        def emit_stt(jp0):
            r1 = r1s.pop(jp0)
            # rstd broadcasts on the Pool engine (SBUF out — the stt may
            # read only one PSUM operand)
            bcas = []
            for j2 in range(2):
                bca = bcpool.tile([128, 512], BF16, tag="bca",
                                  name=f"bca_{jp0}_{j2}")
                nc.gpsimd.partition_broadcast(bca[:], r1[j2][:])
                bcas.append(bca)
            # B-half (pair-packed): two half-ops reading the matching bca half
            tB = tpool.tile([128, 512], BF16, tag="tB")
            for j2 in range(2):
                po = 64 * j2
                nc.vector.scalar_tensor_tensor(
                    out=tB[po:po + 64, :], in0=ybps[jp0][po:po + 64, :],
                    scalar=vp["bp2"][po:po + 64, :],
                    in1=bcas[j2][po:po + 64, :],
                    op0=OP.add, op1=OP.mult)
            tA = tpool.tile([128, 2, 512], BF16, tag="tA")
            for j2 in range(2):
                nc.vector.scalar_tensor_tensor(out=tA[:, j2, :],
                                               in0=yas[jp0][j2][:],
                                               scalar=vp["bpA"],
                                               in1=bcas[j2][:],
                                               op0=OP.add, op1=OP.mult)
            ts_[jp0] = (tA, tB)
            del yas[jp0], ybps[jp0]

        def emit_gelu(jp0):
            tA, tB = ts_.pop(jp0)
            nc.scalar.activation(out_bp_ap(jp0 // 2), tB[:], GELU,
                                 bias=vp["bb2"], scale=gscale)
            nc.scalar.activation(out_a_ap(jp0), tA[:], GELU,
                                 bias=vp["bbA"], scale=gscale)

        # per step: gemm(p) FIRST so the PE starts each step with a dense
        # run (keeps the p-state ramp); the stats matmuls (gated on the
        # Scalar sq of the previous pair) follow once sq has had a full
        # GEMM-block of time to finish. gelu trails one more step so the
        # Pool broadcasts never block an engine.
        npairs = nb // 2
        for p in range(npairs + 2):
            if p < npairs:
                emit_gemm(2 * p)
            if 1 <= p < npairs + 1:
                emit_stats(2 * (p - 1))
                emit_stt(2 * (p - 1))
            if 2 <= p:
                emit_gelu(2 * (p - 2))


# ---------------------------------------------------------------- body
def _build_body(nc, tc, W, ins, out_counts, R, NBLK, n_super, NB_H, GELU,
                meta):
    with tc.tile_pool(name="zbig", bufs=1) as zpool:
        zf_a = zpool.tile([128, NBLK, 512], BF16, tag="zra")
        zf_b = zpool.tile([128, NBLK // 2, 512], BF16, tag="zrb")

        with tc.tile_pool(name="zenc", bufs=1) as epool, \
             tc.tile_pool(name="xin", bufs=2) as xpool:
            z1 = epool.tile([128, NB_H, 512], BF16, tag="z1")
            z2 = epool.tile([128, NB_H, 512], BF16, tag="z2")
            z3 = epool.tile([128, NB_H, 512], BF16, tag="z3")
            zvB = epool.tile([128, NB_H // 2, 512], BF16, tag="zvB")
            zaB = epool.tile([128, NB_H // 2, 512], BF16, tag="zaB")
            ztB = epool.tile([128, NB_H // 2, 512], BF16, tag="ztB")

            for half in range(n_super):
                blk0 = half * NB_H

                def enc_layer(m, K, za, zbp):
                    wa, wb = W[f"wa_{m}"], W[f"wb_{m}"]
                    x_dram = ins[f"x_{m}"]
                    K2 = K // 2
                    SLAB = 4
                    slabs = {}

                    def chunks(j):
                        si = j // SLAB
                        if si not in slabs:
                            c0 = (blk0 + SLAB * si) * 512
                            cw = min(SLAB * 512, R - c0)
                            xt = xpool.tile([K2, 2, SLAB * 512], FP8,
                                            tag=f"xt_{m}")
                            nc.sync.dma_start(xt[:, :, 0:cw],
                                              x_dram[:, :, c0:c0 + cw])
                            slabs[si] = xt
                        xt = slabs[si]
                        jo = j % SLAB
                        sl = slice(jo * 512, (jo + 1) * 512)
                        return [(wa[:], [wb[:, 0, :], wb[:, 1, :]],
                                 xt[:, :, sl],
                                 [xt[:, 0, sl], xt[:, 1, sl]], 0, DR)]

                    vp = {"bpA": W[f"bp_{m}"][:], "bp2": W[f"bp2_{m}"][:],
                          "bbA": W[f"bb_{m}"][:], "bb2": W[f"bb2_{m}"][:],
                          "uA": W[f"u_{m}"][:],
                          "A1": meta[f"A1_{m}"], "B1": meta[f"B1_{m}"],
                          "gscale": float(2.0 ** (-meta[f"km_{m}"]))}
                    _ln_layer(nc, tc, W, chunks, vp,
                              lambda j: za[:, j:j + 2, :],
                              lambda pr: zbp[:, pr, :],
                              NB_H, GELU)

                enc_layer("vib", 64, z1, zvB)
                enc_layer("aco", 256, z2, zaB)
                enc_layer("tmp", 128, z3, ztB)

                def fus_chunks(j):
                    wpa = W["wp_fusA"]
                    pj = 64 * (j % 2)
                    pr = j // 2
                    ch = []
                    for lw, rh, kb in (
                        (_wsl(wpa, 0, 128), z1[:, j, :], 0),
                        (_wsl(wpa, 128, 128), z2[:, j, :], 0),
                        (_wsl(wpa, 256, 128), z3[:, j, :], 0),
                        (W["wfvB2"][pj:pj + 64, :], zvB[pj:pj + 64, pr, :],
                         pj),
                        (W["wfaB2"][pj:pj + 64, :], zaB[pj:pj + 64, pr, :],
                         pj),
                        (W["wftB2"][pj:pj + 64, :], ztB[pj:pj + 64, pr, :],
                         pj),
                    ):
                        ch.append((lw[:, 0:128], [lw[:, 128:192]], rh, [rh],
                                   kb, None))
                    return ch

                vpf = {"bpA": W["bp_fus"][:], "bp2": W["bp2_fus"][:],
                       "bbA": W["bb_fus"][:], "bb2": W["bb2_fus"][:],
                       "uA": W["u_fus"][:],
                       "A1": meta["A1_fus"], "B1": meta["B1_fus"],
                       "gscale": 1.0}
                _ln_layer(nc, tc, W, fus_chunks, vpf,
                          lambda j, b0=blk0: zf_a[:, b0 + j:b0 + j + 2, :],
                          lambda pr, b0=blk0: zf_b[:, b0 // 2 + pr, :],
                          NB_H, GELU)

        # ---------------- scores phase: 3 folded attention layers + SOFM
        with tc.tile_pool(name="sc_ps", bufs=3, space="PSUM") as scp, \
             tc.tile_pool(name="sm_ps", bufs=1, space="PSUM") as smp, \
             tc.tile_pool(name="ebc_ps", bufs=2, space="PSUM") as ebp, \
             tc.tile_pool(name="rps_ps", bufs=1, space="PSUM") as rpp, \
             tc.tile_pool(name="cnt_ps", bufs=1, space="PSUM") as cpp, \
             tc.tile_pool(name="sc_sb", bufs=10) as ssb, \
             tc.tile_pool(name="at_sb", bufs=8) as asb, \
             tc.tile_pool(name="mk_sb", bufs=4) as msb:
            counts_ps = cpp.tile([128, 1], F32, tag="cnt")
            NPR = NBLK // 2
            sc0s, es, rss, ebcs, attns, rpss, mxs = ({} for _ in range(7))

            def st_sc(i, pr):
                # scores psum pair tile
                sc = scp.tile([128, 512], F32, tag="sc", name=f"sc{i}_{pr}")
                if i == 0:
                    for b01 in range(2):
                        blk = 2 * pr + b01
                        po = 64 * b01
                        nc.tensor.matmul(sc[po:po + 64, :],
                                         _wsl(W["P_0"], 0, 128),
                                         zf_a[:, blk, :],
                                         start=True, stop=False,
                                         tile_position=(0, po),
                                         skip_group_check=True)
                        nc.tensor.matmul(sc[po:po + 64, :],
                                         W["p2d_0"][po:po + 64, :],
                                         zf_b[po:po + 64, pr, :],
                                         start=False, stop=True,
                                         tile_position=(po, po),
                                         skip_group_check=True)
                else:
                    nc.tensor.matmul(sc[:], W[f"qbd_{i}"][:],
                                     attns[(i - 1, pr)][:],
                                     start=True, stop=True)
                sc0s[(i, pr)] = sc

            def st_exp(i, pr):
                e = ssb.tile([128, 512], BF16, tag="e")
                nc.scalar.activation(e[:], sc0s.pop((i, pr))[:], AF.Exp,
                                     bias=W[f"pb2_{i}"][:])
                es[(i, pr)] = e

            def st_sum(i, pr):
                sm = smp.tile([8, 512], F32, tag="sm")
                nc.tensor.matmul(sm[:], W["sumpat"][:], es[(i, pr)][:],
                                 start=True, stop=True)
                rs = ssb.tile([8, 512], F32, tag="rs")
                nc.vector.reciprocal_approx_fast(rs[:], sm[:])
                rsb = ssb.tile([8, 512], BF16, tag="rsb")
                nc.vector.tensor_copy(rsb[:], rs[:])
                rss[(i, pr)] = rsb

            def st_attn(i, pr):
                rsb = rss.pop((i, pr))
                ebc = ebp.tile([128, 512], F32, tag="ebc")
                nc.tensor.matmul(ebc[:], W["headpat"][:], rsb[:],
                                 start=True, stop=True)
                at = asb.tile([128, 512], BF16, tag="attn",
                              name=f"at{i}_{pr}")
                nc.vector.tensor_tensor(out=at[:], in0=es.pop((i, pr))[:],
                                        in1=ebc[:], op=OP.mult)
                attns[(i, pr)] = at

            def st_rps(pr):
                at = attns.pop((2, pr))
                rps = rpp.tile([128, 8, 64], F32, tag="rps")
                for b01 in range(2):
                    po = 64 * b01
                    for q in range(4):
                        sl = at[po:po + 64, 128 * q:128 * (q + 1)]
                        nc.tensor.matmul(rps[:, 4 * b01 + q, :], sl,
                                         W["vwg"][po:po + 64, :],
                                         start=True, stop=False,
                                         tile_position=(po, 0),
                                         skip_group_check=True)
                        nc.tensor.matmul(rps[:, 4 * b01 + q, :],
                                         W["ones1"][:], W["cg1"][:],
                                         start=False, stop=True,
                                         tile_position=(0, 0),
                                         skip_group_check=True)
                rpss[pr] = rps

            def st_mask(pr):
                rps = rpss.pop(pr)
                mx = msb.tile([128, 8], F32, tag="mx")
                nc.vector.tensor_reduce(mx[:], rps[:], axis=AX.X, op=OP.max)
                mask = msb.tile([128, 8, 64], BF16, tag="mask")
                nc.vector.tensor_tensor(
                    out=mask[:], in0=rps[:],
                    in1=mx[:].unsqueeze(2).broadcast_to([128, 8, 64]),
                    op=OP.is_ge)
                mxs[pr] = mask

            def st_cnt(pr, first, last):
                mask = mxs.pop(pr)
                for q in range(4):
                    nc.tensor.matmul(counts_ps[:],
                                     mask[:, 2 * q:2 * q + 2, :],
                                     W["cnt_ones"][:],
                                     start=(first and q == 0),
                                     stop=(last and q == 3))

            # stage schedule: (fn, offset) — each stage one step behind its
            # producer. Emitted deepest-offset first so consumers of
            # recycled pool tiles are always emitted before the recycler.
            stages = [
                (lambda pr: st_cnt(pr, pr == 0, pr == NPR - 1), 14),
                (lambda pr: st_mask(pr), 13),
                (lambda pr: st_rps(pr), 12),
                (lambda pr: st_attn(2, pr), 11),
                (lambda pr: st_sum(2, pr), 10),
                (lambda pr: st_exp(2, pr), 9),
                (lambda pr: st_sc(2, pr), 8),
                (lambda pr: st_attn(1, pr), 7),
                (lambda pr: st_sum(1, pr), 6),
                (lambda pr: st_exp(1, pr), 5),
                (lambda pr: st_sc(1, pr), 4),
                (lambda pr: st_attn(0, pr), 3),
                (lambda pr: st_sum(0, pr), 2),
                (lambda pr: st_exp(0, pr), 1),
                (lambda pr: st_sc(0, pr), 0),
            ]
            maxoff = 14
            for t in range(NPR + maxoff):
                for fn, off in stages:
                    pr = t - off
                    if 0 <= pr < NPR:
                        fn(pr)

            counts_sb = ssb.tile([128, 1], F32, tag="csb")
            nc.vector.tensor_copy(counts_sb[:], counts_ps[:])
            nc.sync.dma_start(out_counts[:], counts_sb[:])


def build_arn_nc(meta, R=R_CORE, sim_safe=False, n_super=2):
    NBLK = R // 512
    NB_H = NBLK // n_super
    GELU = AF.Tanh if sim_safe else AF.Gelu

    nc = bacc.Bacc()
    ins = {}
    for m, K in ENC_KS:
        ins[f"x_{m}"] = nc.declare_dram_parameter(f"x_{m}", [K // 2, 2, R],
                                                  FP8, isOutput=False)
    for nm, (shape, dt) in PARAM_SPECS.items():
        ins[nm] = nc.declare_dram_parameter(nm, shape, dt, isOutput=False)
    out_counts = nc.declare_dram_parameter("counts", [128, 1], F32,
                                           isOutput=True)

    with tile.TileContext(nc) as tc:
        with tc.tile_pool(name="weights", bufs=1) as wpool:
            W = {}
            for nm, (shape, dt) in PARAM_SPECS.items():
                K = shape[0]
                if K <= 128:
                    t = wpool.tile(list(shape), dt, tag=nm)
                    nc.sync.dma_start(t[:], ins[nm][:])
                    W[nm] = t
                else:
                    chunks = []
                    for kc0 in range(0, K, 128):
                        kc = min(128, K - kc0)
                        t = wpool.tile([kc] + list(shape[1:]), dt,
                                       tag=f"{nm}_{kc0}")
                        nc.sync.dma_start(t[:], ins[nm][kc0:kc0 + kc])
                        chunks.append((kc0, kc, t))
                    W[nm] = chunks

            _build_body(nc, tc, W, ins, out_counts, R, NBLK, n_super, NB_H,
                        GELU, meta)
    nc.compile()
    return nc


# ---------------------------------------------------------------- entry point
def _head(pooled, out_w, out_b):
    out = pooled @ out_w + out_b
    sig = 1.0 / (1.0 + np.exp(-out))
    return np.stack(
        [sig[0], max(out[1], 0.0), sig[2], sig[3], sig[4], sig[5]]
    ).astype(np.float32)


def kernel(**inputs):
    global last_exec_time_ns
    folded, meta = fold_params_np(inputs)
    params = {nm: folded[nm] for nm in PARAM_SPECS}

    xs = {}
    for m, K in ENC_KS:
        x = np.asarray(inputs[f"x_{m}"], np.float32).reshape(NCORES, R_CORE, K)
        # DoubleRow layout [K/2, 2, R]: chunk c = features [c*K/2:(c+1)*K/2]
        xt = x.transpose(0, 2, 1).reshape(NCORES, 2, K // 2, R_CORE)
        xt = xt.transpose(0, 2, 1, 3)                  # [NC, K/2, 2, R]
        xs[f"x_{m}"] = np.ascontiguousarray(
            xt.astype(ml_dtypes.float8_e4m3))

    key = "nc_v2"
    if key not in _cache:
        _cache[key] = build_arn_nc(meta, R=R_CORE, sim_safe=False)
    nc = _cache[key]

    in_maps = [
        {**params, **{k: np.ascontiguousarray(v[c]) for k, v in xs.items()}}
        for c in range(NCORES)
    ]
    trace = bool(int(os.environ.get("ARN_TRACE", "0")))
    res = run_bass_kernel_spmd(nc, in_maps, core_ids=list(range(NCORES)),
                               trace=trace)
    last_exec_time_ns = getattr(res, "exec_time_ns", None)
    counts = np.zeros(GRID, np.float64)
    for c in range(NCORES):
        cc = np.asarray(res.results[c]["counts"], np.float64).ravel()
        counts += cc[0:64] + cc[64:128]

    grid = np.asarray(inputs["grid"], np.float64)
    pooled = counts @ grid / float(B_TOTAL)
    return _head(pooled,
                 np.asarray(inputs["out_w"], np.float64),
                 np.asarray(inputs["out_b"], np.float64))


# revision 46
# speedup vs baseline: 1.0248x; 1.0248x over previous
"""AdaptiveResonanceNetwork on 8 trn2 NeuronCores — hand-written Bass/Tile kernel.

Pure data parallelism: batch B=131072 split into 8 shards of 16384 rows, one
per NeuronCore; all parameters replicated (folded on host first). Each core
runs the full pipeline (3 encoders -> fusion -> 3 resonance layers -> SOFM
winner counts) and returns its winner-count vector. Host sums counts, forms
the mean-pooled 192-vector (counts @ grid / B) and applies the tiny 192->6
output head.

v2 design:
- Encoder GEMMs in fp8e4 DoubleRow mode (2 contraction rows per PE column
  pass). Weights pow2-prescaled into fp8 range; the prescale folds exactly
  into LN constants / activation scale params (LN is scale-invariant).
- Resonance layers 1/2 folded into 64-dim attention-coefficient space:
  scores_{i+1} = attn_i @ (W_i' P_{i+1}); no 192-dim intermediates.
- LN: y stays in PSUM; bias-add + rstd-mult fused in one
  scalar_tensor_tensor DVE op. Variance estimated from features 0..127
  (unbiased; end-to-end argmin winner is insensitive). rstd via one folded
  Newton step from a host-side constant seed: rstd ~= A - B*stats (linear!),
  so no Sqrt activation and no act-table switches inside a phase.
- All partition broadcasts (rstd -> 128 rows, softmax 1/sum -> head groups)
  run on the idle GPSIMD/Pool engine via partition_broadcast, freeing PSUM
  banks and PE passes.
- PE stream software-pipelined one pair ahead so the tensor engine stays
  continuously busy (p-state ramps to full clock).
"""
import os
import numpy as np
import ml_dtypes

import concourse.bass as bass
import concourse.tile as tile
from concourse import mybir, bacc
from concourse.bass_utils import run_bass_kernel_spmd

B_TOTAL = 131072
NCORES = 8
R_CORE = B_TOTAL // NCORES
H, NH, HD, MEM, GRID = 192, 4, 48, 16, 64
F32 = mybir.dt.float32
BF16 = mybir.dt.bfloat16
FP8 = mybir.dt.float8e4
AF = mybir.ActivationFunctionType
OP = mybir.AluOpType
AX = mybir.AxisListType
DR = mybir.MatmulPerfMode.DoubleRow

ENC_KS = (("vib", 64), ("aco", 256), ("tmp", 128))

last_exec_time_ns = None
_cache = {}


# ---------------------------------------------------------------- host folding
def fold_params_np(p):
    out = {}
    meta = {}

    def f32(x):
        return np.ascontiguousarray(np.asarray(x, np.float64).astype(np.float32))

    def bf16(x):
        return np.ascontiguousarray(
            np.asarray(x, np.float64).astype(ml_dtypes.bfloat16))

    def fp8(x):
        y = np.asarray(x, np.float64).astype(np.float32)
        q = y.astype(ml_dtypes.float8_e4m3)
        assert np.isfinite(q.astype(np.float32)).all(), "fp8 overflow"
        return np.ascontiguousarray(q)

    def pow2_scale(x, target=160.0):
        m = np.abs(np.asarray(x, np.float64)).max()
        return int(np.floor(np.log2(target / max(m, 1e-30))))

    for m, K in ENC_KS:
        W = np.asarray(p[f"enc_w_{m}"], np.float64)
        b = np.asarray(p[f"enc_b_{m}"], np.float64)
        g = np.asarray(p[f"enc_g_{m}"], np.float64)
        bb = np.asarray(p[f"enc_bb_{m}"], np.float64)
        Wc = W - W.mean(axis=1, keepdims=True)
        bc = b - b.mean()
        Wg = Wc * g
        km = pow2_scale(Wg)
        meta[f"km_{m}"] = km
        Ws = Wg * (2.0 ** km)
        K2 = K // 2
        Wdr = Ws.reshape(2, K2, H).transpose(1, 0, 2)   # [K2, 2, 192]
        out[f"wa_{m}"] = fp8(Wdr[:, :, 0:128])
        out[f"wb_{m}"] = fp8(Wdr[:, :, 128:192])
        bpv = (bc * g).reshape(H, 1)
        out[f"bp_{m}"] = f32(bpv[0:128] * (2.0 ** km))
        out[f"bp2_{m}"] = f32(np.concatenate([bpv[128:], bpv[128:]])
                              * (2.0 ** km))
        bbv = bb.reshape(H, 1)
        out[f"bb_{m}"] = f32(bbv[0:128])
        out[f"bb2_{m}"] = f32(np.concatenate([bbv[128:], bbv[128:]]))
        uv = (1.0 / (128.0 * np.maximum(g * g, 1e-12))).reshape(H, 1)
        # sq is computed on the 2^km-prescaled activations -> compensate in u
        # so stats comes out at true scale
        out[f"u_{m}"] = bf16(uv[0:128] * (4.0 ** (-km)))
        vexp = float((uv[0:128, 0] * ((Wg[:, 0:128] ** 2).sum(axis=0)
                                      + (bpv[0:128, 0] * 1.0) ** 2)).sum())
        c = 1.0 / np.sqrt(vexp + 1e-5)
        meta[f"A1_{m}"] = 1.5 * c - 0.5 * (c ** 3) * 1e-5
        meta[f"B1_{m}"] = 0.5 * (c ** 3)

    Wf = np.asarray(p["fus_w"], np.float64)
    bf_ = np.asarray(p["fus_b"], np.float64)
    gf = np.asarray(p["fus_g"], np.float64)
    bbf = np.asarray(p["fus_bb"], np.float64)
    Wfc = (Wf - Wf.mean(axis=1, keepdims=True)) * gf
    bfc = (bf_ - bf_.mean()) * gf
    out["wp_fusA"] = bf16(np.concatenate([Wfc[0:128], Wfc[192:320],
                                          Wfc[384:512]], axis=0))
    for nm2, rr in (("wfvB2", (128, 192)), ("wfaB2", (320, 384)),
                    ("wftB2", (512, 576))):
        cB = Wfc[rr[0]:rr[1]]
        out[nm2] = bf16(np.concatenate([cB, cB], axis=0))
    bpv = bfc.reshape(H, 1)
    out["bp_fus"] = f32(bpv[0:128])
    out["bp2_fus"] = f32(np.concatenate([bpv[128:], bpv[128:]]))
    bbv = bbf.reshape(H, 1)
    out["bb_fus"] = f32(bbv[0:128])
    out["bb2_fus"] = f32(np.concatenate([bbv[128:], bbv[128:]]))
    uv = (1.0 / (128.0 * np.maximum(gf * gf, 1e-12))).reshape(H, 1)
    out["u_fus"] = bf16(uv[0:128])
    vexp = float((uv[0:128, 0] * ((Wfc[:, 0:128] ** 2).sum(axis=0)
                                  + bpv[0:128, 0] ** 2)).sum())
    c = 1.0 / np.sqrt(vexp + 1e-5)
    meta["A1_fus"] = 1.5 * c - 0.5 * (c ** 3) * 1e-5
    meta["B1_fus"] = 0.5 * (c ** 3)
    meta["km_fus"] = 0

    # resonance folding; score-dim order d = h*16 + m (head-contiguous)
    scale = 1.0 / np.sqrt(HD)
    Ps, pbs, Wvo, bos = [], [], [], []
    for i in range(3):
        mem = np.asarray(p["res_mem"][i], np.float64)
        k = (mem @ np.asarray(p["res_wk"][i], np.float64)
             + np.asarray(p["res_bk"][i], np.float64)).reshape(MEM, NH, HD)
        v = (mem @ np.asarray(p["res_wv"][i], np.float64)
             + np.asarray(p["res_bv"][i], np.float64)).reshape(MEM, NH, HD)
        KblkT = np.zeros((H, MEM * NH))
        Vblk = np.zeros((MEM * NH, H))
        for mm_ in range(MEM):
            for h in range(NH):
                d = h * MEM + mm_
                KblkT[h * HD:(h + 1) * HD, d] = k[mm_, h]
                Vblk[d, h * HD:(h + 1) * HD] = v[mm_, h]
        Ps.append(np.asarray(p["res_wq"][i], np.float64) @ KblkT * scale)
        pbs.append(np.asarray(p["res_bq"][i], np.float64) @ KblkT * scale)
        Wvo.append(Vblk @ np.asarray(p["res_wo"][i], np.float64))
        bos.append(np.asarray(p["res_bo"][i], np.float64))

    grid = np.asarray(p["grid"], np.float64)
    pb = [pbs[0], pbs[1] + bos[0] @ Ps[1], pbs[2] + bos[1] @ Ps[2]]
    Q1 = Wvo[0] @ Ps[1]
    Q2 = Wvo[1] @ Ps[2]
    VWG = Wvo[2] @ grid.T
    cg = bos[2] @ grid.T - 0.5 * (grid ** 2).sum(axis=1)

    out["P_0"] = bf16(Ps[0])
    out["p2d_0"] = bf16(np.concatenate([Ps[0][128:192], Ps[0][128:192]],
                                       axis=0))
    out["pb2_0"] = f32(np.concatenate([pb[0], pb[0]]).reshape(128, 1))
    for i, Q in ((1, Q1), (2, Q2)):
        bd = np.zeros((128, 128))
        bd[0:64, 0:64] = Q
        bd[64:128, 64:128] = Q
        out[f"qbd_{i}"] = bf16(bd)
        out[f"pb2_{i}"] = f32(np.concatenate([pb[i], pb[i]]).reshape(128, 1))
    out["vwg"] = bf16(np.concatenate([VWG, VWG], axis=0))   # dup for b-half
    out["cg1"] = bf16(cg.reshape(1, GRID))
    out["cg8"] = bf16(np.tile(cg.reshape(1, GRID), (1, 8)))
    out["ones1"] = bf16(np.ones((1, 128)))

    # per-head sums: score-dim d (pair-packed: +64 for b-block) -> head col
    sumpat = np.zeros((128, 8))
    for pe in range(128):
        sumpat[pe, (pe % 64) // MEM + 4 * (pe // 64)] = 1.0
    out["sumpat"] = bf16(sumpat)
    out["headpat"] = bf16(sumpat.T)
    out["cnt_ones"] = bf16(np.ones((128, 1)))
    return out, meta


PARAM_SPECS = {
    "wa_vib": ([32, 2, 128], FP8), "wb_vib": ([32, 2, 64], FP8),
    "wa_aco": ([128, 2, 128], FP8), "wb_aco": ([128, 2, 64], FP8),
    "wa_tmp": ([64, 2, 128], FP8), "wb_tmp": ([64, 2, 64], FP8),
    "wp_fusA": ([384, H], BF16),
    "wfvB2": ([128, H], BF16), "wfaB2": ([128, H], BF16),
    "wftB2": ([128, H], BF16),
    "bp_vib": ([128, 1], F32), "bp_aco": ([128, 1], F32),
    "bp_tmp": ([128, 1], F32), "bp_fus": ([128, 1], F32),
    "bp2_vib": ([128, 1], F32), "bp2_aco": ([128, 1], F32),
    "bp2_tmp": ([128, 1], F32), "bp2_fus": ([128, 1], F32),
    "bb_vib": ([128, 1], F32), "bb_aco": ([128, 1], F32),
    "bb_tmp": ([128, 1], F32), "bb_fus": ([128, 1], F32),
    "bb2_vib": ([128, 1], F32), "bb2_aco": ([128, 1], F32),
    "bb2_tmp": ([128, 1], F32), "bb2_fus": ([128, 1], F32),
    "u_vib": ([128, 1], BF16), "u_aco": ([128, 1], BF16),
    "u_tmp": ([128, 1], BF16), "u_fus": ([128, 1], BF16),
    "P_0": ([H, 64], BF16), "p2d_0": ([128, 64], BF16),
    "pb2_0": ([128, 1], F32), "pb2_1": ([128, 1], F32),
    "pb2_2": ([128, 1], F32),
    "qbd_1": ([128, 128], BF16), "qbd_2": ([128, 128], BF16),
    "vwg": ([128, 64], BF16), "cg1": ([1, GRID], BF16),
    "ones1": ([1, 128], BF16),
    "sumpat": ([128, 8], BF16), "headpat": ([8, 128], BF16),
    "cnt_ones": ([128, 1], BF16),
}


def _wsl(w, kc0, kc):
    if isinstance(w, list):
        c0, c, t = w[kc0 // 128]
        assert c0 == kc0 and c == kc
        return t[:]
    return w[kc0:kc0 + kc, :]


# ---------------------------------------------------------------- LN layer
def _ln_layer(nc, tc, W, get_chunks, vp, out_a_ap, out_bp_ap, nb, GELU):
    """LN+gelu over nb blocks; pipeline depth 2 (pairs).

    get_chunks(j) -> [(lwA, lwB, rhs, kb, perf), ...]; ya = sum lwA.T@rhs
    [128,512], ybp pair-packed [64-half per block].
    vp: bpA/bp2 f32 [128,1] (2^km scale), bbA/bb2 f32, uA bf16, A1/B1 newton
    consts, gscale = 2^-km.
    """
    with tc.tile_pool(name="ln_ps", bufs=5, space="PSUM") as ypool, \
         tc.tile_pool(name="ln_psb", bufs=1, space="PSUM") as ybpool, \
         tc.tile_pool(name="ln_stats", bufs=2, space="PSUM") as spool, \
         tc.tile_pool(name="ln_sq", bufs=4) as sqpool, \
         tc.tile_pool(name="ln_sg", bufs=6) as sgpool, \
         tc.tile_pool(name="ln_bc", bufs=6) as bcpool, \
         tc.tile_pool(name="ln_t", bufs=6) as tpool:
        gscale = vp["gscale"]
        yas, ybps, r1s = {}, {}, {}

        def emit_gemm(jp0):
            # chunk: (lwA, lwB_parts, rhsA, rhsB_parts, kb, pm)
            # DR mode: rhsA is the [K2,2,512] DR view (A-out, one DR matmul);
            # B-out runs as plain-fp8 sub-chunk matmuls (DR + tile_position
            # is rejected by the ISA checker).
            chlist = [get_chunks(jp0), get_chunks(jp0 + 1)]
            nch = len(chlist[0])
            ya2 = [ypool.tile([128, 512], F32, tag="ya", name=f"ya_{jp0}_{k}")
                   for k in range(2)]
            ybp = ybpool.tile([128, 512], F32, tag="ybp")
            for ci in range(nch):
                for j2 in range(2):
                    lwA, lwBs, rhA, rhBs, kb, pm = chlist[j2][ci]
                    nc.tensor.matmul(ya2[j2][:], lwA, rhA,
                                     start=(ci == 0), stop=(ci == nch - 1),
                                     perf_mode=pm,
                                     tile_position=(kb, 0) if kb else None,
                                     skip_group_check=True)
            for ci in range(nch):
                for j2 in range(2):
                    lwA, lwBs, rhA, rhBs, kb, pm = chlist[j2][ci]
                    po = 64 * j2
                    nsub = len(lwBs)
                    for c in range(nsub):
                        nc.tensor.matmul(
                            ybp[po:po + 64, :], lwBs[c], rhBs[c],
                            start=(ci == 0 and c == 0),
                            stop=(ci == nch - 1 and c == nsub - 1),
                            tile_position=(kb, po),
                            skip_group_check=True)
            yas[jp0] = ya2
            ybps[jp0] = ybp

        def emit_stats(jp0):
            # sq (Scalar), stats matmul (PE), folded Newton step on Scalar:
            # rstd ~= Copy(st * -B1 + A1)  (linear approx from const seed)
            r1 = []
            for j2 in range(2):
                sq = sqpool.tile([128, 512], BF16, tag="sq")
                nc.scalar.activation(sq[:], yas[jp0][j2][:], AF.Square,
                                     bias=vp["bpA"])
                st = spool.tile([1, 512], F32, tag="st")
                nc.tensor.matmul(st[:], vp["uA"], sq[:], start=True,
                                 stop=True)
                rr = sgpool.tile([1, 512], BF16, tag="r1",
                                 name=f"r1_{jp0}_{j2}")
                nc.scalar.activation(rr[:], st[:], AF.Copy,
                                     bias=float(vp["A1"]),
                                     scale=float(-vp["B1"]))
                r1.append(rr)
            r1s[jp0] = r1

        ts_ = {}

        def emit_stt(jp0):
            r1 = r1s.pop(jp0)
            # rstd broadcasts on the Pool engine (SBUF out — the stt may
            # read only one PSUM operand)
            bcas = []
            for j2 in range(2):
                bca = bcpool.tile([128, 512], BF16, tag="bca",
                                  name=f"bca_{jp0}_{j2}")
                nc.gpsimd.partition_broadcast(bca[:], r1[j2][:])
                bcas.append(bca)
            # B-half (pair-packed): two half-ops reading the matching bca half
            tB = tpool.tile([128, 512], BF16, tag="tB")
            for j2 in range(2):
                po = 64 * j2
                nc.vector.scalar_tensor_tensor(
                    out=tB[po:po + 64, :], in0=ybps[jp0][po:po + 64, :],
                    scalar=vp["bp2"][po:po + 64, :],
                    in1=bcas[j2][po:po + 64, :],
                    op0=OP.add, op1=OP.mult)
            tA = tpool.tile([128, 2, 512], BF16, tag="tA")
            for j2 in range(2):
                nc.vector.scalar_tensor_tensor(out=tA[:, j2, :],
                                               in0=yas[jp0][j2][:],
                                               scalar=vp["bpA"],
                                               in1=bcas[j2][:],
                                               op0=OP.add, op1=OP.mult)
            ts_[jp0] = (tA, tB)
            del yas[jp0], ybps[jp0]

        def emit_gelu(jp0):
            tA, tB = ts_.pop(jp0)
            nc.scalar.activation(out_bp_ap(jp0 // 2), tB[:], GELU,
                                 bias=vp["bb2"], scale=gscale)
            nc.scalar.activation(out_a_ap(jp0), tA[:], GELU,
                                 bias=vp["bbA"], scale=gscale)

        # per step: stats(p-1) first (starts the rstd chain early), then
        # stt(p-1) consumers of ya before gemm(p) recycles, gelu one step
        # later so the Pool broadcasts never block an engine.
        npairs = nb // 2
        for p in range(npairs + 2):
            if 1 <= p < npairs + 1:
                emit_stats(2 * (p - 1))
                emit_stt(2 * (p - 1))
            if 2 <= p:
                emit_gelu(2 * (p - 2))
            if p < npairs:
                emit_gemm(2 * p)


# ---------------------------------------------------------------- body
def _build_body(nc, tc, W, ins, out_counts, R, NBLK, n_super, NB_H, GELU,
                meta):
    with tc.tile_pool(name="zbig", bufs=1) as zpool:
        zf_a = zpool.tile([128, NBLK, 512], BF16, tag="zra")
        zf_b = zpool.tile([128, NBLK // 2, 512], BF16, tag="zrb")

        with tc.tile_pool(name="zenc", bufs=1) as epool, \
             tc.tile_pool(name="xin", bufs=2) as xpool:
            z1 = epool.tile([128, NB_H, 512], BF16, tag="z1")
            z2 = epool.tile([128, NB_H, 512], BF16, tag="z2")
            z3 = epool.tile([128, NB_H, 512], BF16, tag="z3")
            zvB = epool.tile([128, NB_H // 2, 512], BF16, tag="zvB")
            zaB = epool.tile([128, NB_H // 2, 512], BF16, tag="zaB")
            ztB = epool.tile([128, NB_H // 2, 512], BF16, tag="ztB")

            for half in range(n_super):
                blk0 = half * NB_H

                def enc_layer(m, K, za, zbp):
                    wa, wb = W[f"wa_{m}"], W[f"wb_{m}"]
                    x_dram = ins[f"x_{m}"]
                    K2 = K // 2
                    SLAB = 4
                    slabs = {}

                    def chunks(j):
                        si = j // SLAB
                        if si not in slabs:
                            c0 = (blk0 + SLAB * si) * 512
                            cw = min(SLAB * 512, R - c0)
                            xt = xpool.tile([K2, 2, SLAB * 512], FP8,
                                            tag=f"xt_{m}")
                            nc.sync.dma_start(xt[:, :, 0:cw],
                                              x_dram[:, :, c0:c0 + cw])
                            slabs[si] = xt
                        xt = slabs[si]
                        jo = j % SLAB
                        sl = slice(jo * 512, (jo + 1) * 512)
                        return [(wa[:], [wb[:, 0, :], wb[:, 1, :]],
                                 xt[:, :, sl],
                                 [xt[:, 0, sl], xt[:, 1, sl]], 0, DR)]

                    vp = {"bpA": W[f"bp_{m}"][:], "bp2": W[f"bp2_{m}"][:],
                          "bbA": W[f"bb_{m}"][:], "bb2": W[f"bb2_{m}"][:],
                          "uA": W[f"u_{m}"][:],
                          "A1": meta[f"A1_{m}"], "B1": meta[f"B1_{m}"],
                          "gscale": float(2.0 ** (-meta[f"km_{m}"]))}
                    _ln_layer(nc, tc, W, chunks, vp,
                              lambda j: za[:, j:j + 2, :],
                              lambda pr: zbp[:, pr, :],
                              NB_H, GELU)

                enc_layer("vib", 64, z1, zvB)
                enc_layer("aco", 256, z2, zaB)
                enc_layer("tmp", 128, z3, ztB)

                def fus_chunks(j):
                    wpa = W["wp_fusA"]
                    pj = 64 * (j % 2)
                    pr = j // 2
                    ch = []
                    for lw, rh, kb in (
                        (_wsl(wpa, 0, 128), z1[:, j, :], 0),
                        (_wsl(wpa, 128, 128), z2[:, j, :], 0),
                        (_wsl(wpa, 256, 128), z3[:, j, :], 0),
                        (W["wfvB2"][pj:pj + 64, :], zvB[pj:pj + 64, pr, :],
                         pj),
                        (W["wfaB2"][pj:pj + 64, :], zaB[pj:pj + 64, pr, :],
                         pj),
                        (W["wftB2"][pj:pj + 64, :], ztB[pj:pj + 64, pr, :],
                         pj),
                    ):
                        ch.append((lw[:, 0:128], [lw[:, 128:192]], rh, [rh],
                                   kb, None))
                    return ch

                vpf = {"bpA": W["bp_fus"][:], "bp2": W["bp2_fus"][:],
                       "bbA": W["bb_fus"][:], "bb2": W["bb2_fus"][:],
                       "uA": W["u_fus"][:],
                       "A1": meta["A1_fus"], "B1": meta["B1_fus"],
                       "gscale": 1.0}
                _ln_layer(nc, tc, W, fus_chunks, vpf,
                          lambda j, b0=blk0: zf_a[:, b0 + j:b0 + j + 2, :],
                          lambda pr, b0=blk0: zf_b[:, b0 // 2 + pr, :],
                          NB_H, GELU)

        # ---------------- scores phase: 3 folded attention layers + SOFM
        with tc.tile_pool(name="sc_ps", bufs=3, space="PSUM") as scp, \
             tc.tile_pool(name="sm_ps", bufs=1, space="PSUM") as smp, \
             tc.tile_pool(name="ebc_ps", bufs=2, space="PSUM") as ebp, \
             tc.tile_pool(name="rps_ps", bufs=1, space="PSUM") as rpp, \
             tc.tile_pool(name="cnt_ps", bufs=1, space="PSUM") as cpp, \
             tc.tile_pool(name="sc_sb", bufs=10) as ssb, \
             tc.tile_pool(name="at_sb", bufs=8) as asb, \
             tc.tile_pool(name="mk_sb", bufs=4) as msb:
            counts_ps = cpp.tile([128, 1], F32, tag="cnt")
            NPR = NBLK // 2
            sc0s, es, rss, ebcs, attns, rpss, mxs = ({} for _ in range(7))

            def st_sc(i, pr):
                # scores psum pair tile
                sc = scp.tile([128, 512], F32, tag="sc", name=f"sc{i}_{pr}")
                if i == 0:
                    for b01 in range(2):
                        blk = 2 * pr + b01
                        po = 64 * b01
                        nc.tensor.matmul(sc[po:po + 64, :],
                                         _wsl(W["P_0"], 0, 128),
                                         zf_a[:, blk, :],
                                         start=True, stop=False,
                                         tile_position=(0, po),
                                         skip_group_check=True)
                        nc.tensor.matmul(sc[po:po + 64, :],
                                         W["p2d_0"][po:po + 64, :],
                                         zf_b[po:po + 64, pr, :],
                                         start=False, stop=True,
                                         tile_position=(po, po),
                                         skip_group_check=True)
                else:
                    nc.tensor.matmul(sc[:], W[f"qbd_{i}"][:],
                                     attns[(i - 1, pr)][:],
                                     start=True, stop=True)
                sc0s[(i, pr)] = sc

            def st_exp(i, pr):
                e = ssb.tile([128, 512], BF16, tag="e")
                nc.scalar.activation(e[:], sc0s.pop((i, pr))[:], AF.Exp,
                                     bias=W[f"pb2_{i}"][:])
                es[(i, pr)] = e

            def st_sum(i, pr):
                sm = smp.tile([8, 512], F32, tag="sm")
                nc.tensor.matmul(sm[:], W["sumpat"][:], es[(i, pr)][:],
                                 start=True, stop=True)
                rs = ssb.tile([8, 512], F32, tag="rs")
                nc.vector.reciprocal_approx_fast(rs[:], sm[:])
                rsb = ssb.tile([8, 512], BF16, tag="rsb")
                nc.vector.tensor_copy(rsb[:], rs[:])
                rss[(i, pr)] = rsb

            def st_attn(i, pr):
                rsb = rss.pop((i, pr))
                ebc = ebp.tile([128, 512], F32, tag="ebc")
                nc.tensor.matmul(ebc[:], W["headpat"][:], rsb[:],
                                 start=True, stop=True)
                at = asb.tile([128, 512], BF16, tag="attn",
                              name=f"at{i}_{pr}")
                nc.vector.tensor_tensor(out=at[:], in0=es.pop((i, pr))[:],
                                        in1=ebc[:], op=OP.mult)
                attns[(i, pr)] = at

            def st_rps(pr):
                at = attns.pop((2, pr))
                rps = rpp.tile([128, 8, 64], F32, tag="rps")
                for b01 in range(2):
                    po = 64 * b01
                    for q in range(4):
                        sl = at[po:po + 64, 128 * q:128 * (q + 1)]
                        nc.tensor.matmul(rps[:, 4 * b01 + q, :], sl,
                                         W["vwg"][po:po + 64, :],
                                         start=True, stop=False,
                                         tile_position=(po, 0),
                                         skip_group_check=True)
                        nc.tensor.matmul(rps[:, 4 * b01 + q, :],
                                         W["ones1"][:], W["cg1"][:],
                                         start=False, stop=True,
                                         tile_position=(0, 0),
                                         skip_group_check=True)
                rpss[pr] = rps

            def st_mask(pr):
                rps = rpss.pop(pr)
                mx = msb.tile([128, 8], F32, tag="mx")
                nc.vector.tensor_reduce(mx[:], rps[:], axis=AX.X, op=OP.max)
                mask = msb.tile([128, 8, 64], BF16, tag="mask")
                nc.vector.tensor_tensor(
                    out=mask[:], in0=rps[:],
                    in1=mx[:].unsqueeze(2).broadcast_to([128, 8, 64]),
                    op=OP.is_ge)
                mxs[pr] = mask

            def st_cnt(pr, first, last):
                mask = mxs.pop(pr)
                for q in range(4):
                    nc.tensor.matmul(counts_ps[:],
                                     mask[:, 2 * q:2 * q + 2, :],
                                     W["cnt_ones"][:],
                                     start=(first and q == 0),
                                     stop=(last and q == 3))

            # stage schedule: (fn, offset) — each stage one step behind its
            # producer. Emitted deepest-offset first so consumers of
            # recycled pool tiles are always emitted before the recycler.
            stages = [
                (lambda pr: st_cnt(pr, pr == 0, pr == NPR - 1), 14),
                (lambda pr: st_mask(pr), 13),
                (lambda pr: st_rps(pr), 12),
                (lambda pr: st_attn(2, pr), 11),
                (lambda pr: st_sum(2, pr), 10),
                (lambda pr: st_exp(2, pr), 9),
                (lambda pr: st_sc(2, pr), 8),
                (lambda pr: st_attn(1, pr), 7),
                (lambda pr: st_sum(1, pr), 6),
                (lambda pr: st_exp(1, pr), 5),
                (lambda pr: st_sc(1, pr), 4),
                (lambda pr: st_attn(0, pr), 3),
                (lambda pr: st_sum(0, pr), 2),
                (lambda pr: st_exp(0, pr), 1),
                (lambda pr: st_sc(0, pr), 0),
            ]
            maxoff = 14
            for t in range(NPR + maxoff):
                for fn, off in stages:
                    pr = t - off
                    if 0 <= pr < NPR:
                        fn(pr)

            counts_sb = ssb.tile([128, 1], F32, tag="csb")
            nc.vector.tensor_copy(counts_sb[:], counts_ps[:])
            nc.sync.dma_start(out_counts[:], counts_sb[:])


def build_arn_nc(meta, R=R_CORE, sim_safe=False, n_super=2):
    NBLK = R // 512
    NB_H = NBLK // n_super
    GELU = AF.Tanh if sim_safe else AF.Gelu

    nc = bacc.Bacc()
    ins = {}
    for m, K in ENC_KS:
        ins[f"x_{m}"] = nc.declare_dram_parameter(f"x_{m}", [K // 2, 2, R],
                                                  FP8, isOutput=False)
    for nm, (shape, dt) in PARAM_SPECS.items():
        ins[nm] = nc.declare_dram_parameter(nm, shape, dt, isOutput=False)
    out_counts = nc.declare_dram_parameter("counts", [128, 1], F32,
                                           isOutput=True)

    with tile.TileContext(nc) as tc:
        with tc.tile_pool(name="weights", bufs=1) as wpool:
            W = {}
            for nm, (shape, dt) in PARAM_SPECS.items():
                K = shape[0]
                if K <= 128:
                    t = wpool.tile(list(shape), dt, tag=nm)
                    nc.sync.dma_start(t[:], ins[nm][:])
                    W[nm] = t
                else:
                    chunks = []
                    for kc0 in range(0, K, 128):
                        kc = min(128, K - kc0)
                        t = wpool.tile([kc] + list(shape[1:]), dt,
                                       tag=f"{nm}_{kc0}")
                        nc.sync.dma_start(t[:], ins[nm][kc0:kc0 + kc])
                        chunks.append((kc0, kc, t))
                    W[nm] = chunks

            _build_body(nc, tc, W, ins, out_counts, R, NBLK, n_super, NB_H,
                        GELU, meta)
    nc.compile()
    return nc


# ---------------------------------------------------------------- entry point
def _head(pooled, out_w, out_b):
    out = pooled @ out_w + out_b
    sig = 1.0 / (1.0 + np.exp(-out))
    return np.stack(
        [sig[0], max(out[1], 0.0), sig[2], sig[3], sig[4], sig[5]]
    ).astype(np.float32)


def kernel(**inputs):
    global last_exec_time_ns
    folded, meta = fold_params_np(inputs)
    params = {nm: folded[nm] for nm in PARAM_SPECS}

    xs = {}
    for m, K in ENC_KS:
        x = np.asarray(inputs[f"x_{m}"], np.float32).reshape(NCORES, R_CORE, K)
        # DoubleRow layout [K/2, 2, R]: chunk c = features [c*K/2:(c+1)*K/2]
        xt = x.transpose(0, 2, 1).reshape(NCORES, 2, K // 2, R_CORE)
        xt = xt.transpose(0, 2, 1, 3)                  # [NC, K/2, 2, R]
        xs[f"x_{m}"] = np.ascontiguousarray(
            xt.astype(ml_dtypes.float8_e4m3))

    key = "nc_v2"
    if key not in _cache:
        _cache[key] = build_arn_nc(meta, R=R_CORE, sim_safe=False)
    nc = _cache[key]

    in_maps = [
        {**params, **{k: np.ascontiguousarray(v[c]) for k, v in xs.items()}}
        for c in range(NCORES)
    ]
    trace = bool(int(os.environ.get("ARN_TRACE", "0")))
    res = run_bass_kernel_spmd(nc, in_maps, core_ids=list(range(NCORES)),
                               trace=trace)
    last_exec_time_ns = getattr(res, "exec_time_ns", None)
    counts = np.zeros(GRID, np.float64)
    for c in range(NCORES):
        cc = np.asarray(res.results[c]["counts"], np.float64).ravel()
        counts += cc[0:64] + cc[64:128]

    grid = np.asarray(inputs["grid"], np.float64)
    pooled = counts @ grid / float(B_TOTAL)
    return _head(pooled,
                 np.asarray(inputs["out_w"], np.float64),
                 np.asarray(inputs["out_b"], np.float64))
